# revision 26
# baseline (speedup 1.0000x reference)
"""Bass/Trainium2 kernel for ComplexUpSampling2D (2x bilinear, half-pixel centers).

Input:  (16, 128, 128, 128) f32  (B, H, W, C)
Output: (16, 256, 256, 128) f32

Math (per axis, factor 2, half-pixel, with edge clamp):
  out[2i]   = 0.25*in[i-1] + 0.75*in[i]    (in[-1] clamped to in[0])
  out[2i+1] = 0.75*in[i]   + 0.25*in[i+1]  (in[n] clamped to in[n-1])

Strategy (pure data-parallel over batch: 2 images per core on 8 cores):
  - SBUF layout: partitions = H (128), free dim = W*C (16384) per image.
    Each image is loaded ONCE into a resident tile with a duplicated C-block
    on each end (the W edge clamp), so every F-wide compute chunk slices a
    uniform (F + 2C)-wide halo'd window out of it - no per-chunk input DMAs
    and minimal HBM read traffic.
  - H-interp mixes partitions -> done on the TensorEngine as qE = M_E @ cur,
    qO = M_O @ cur with banded 128x128 fp32 matrices (two nonzeros per row:
    3/16 and 1/16, edge rows 4/16) that also fold in the /16 normalization
    and the H edge clamp. This avoids partition-shifted DMAs entirely (both
    DGE paths degenerate to one-descriptor-per-partition on a single DMA
    engine for partition-misaligned transfers).
  - PSUM results are copied to SBUF by the scalar engine (DMA cannot read
    PSUM, and the W-stage reads each q twice so it cannot stay in PSUM).
  - W-interp mixes w-neighbors C elements apart in the free dim -> fused
    scalar_tensor_tensor DVE ops on shifted access patterns (q = row/4):
        out[., even w] = 3*q[j] + q[j-1]
        out[., odd  w] = 3*q[j] + q[j+1]
  - Both output row phases are written into one SBUF tile and stored with a
    single DMA per chunk (DRAM rows 2p, 2p+1 are per-partition row pairs).
  - Raw bass with explicit standalone wait_ge ops (the walrus codegen on
    this run path supports only one embedded sync-wait per instruction).
  - DMA semaphores are lane-split so that every wait threshold equals
    16 x (all DMAs ever issued on that semaphore at that point): a DMA's 16
    completion increments are spread across engines, so a shared cumulative
    wait could otherwise be satisfied by partial credit from a later
    in-flight DMA on the same semaphore.
  - All semaphores are reset to zero at the end behind a finish barrier so
    the NEFF can be re-executed.
"""

from contextlib import ExitStack

import numpy as np

import concourse.bass as bass
from concourse import mybir
from concourse.bass_utils import run_bass_kernel_spmd

B, H, W, C = 16, 128, 128, 128
NCORES = 8
BS = B // NCORES          # images per core
WC = W * C                # 16384 free elements per input row
F = 1024                  # chunk width (input free elements) = 8 w-blocks
NW = F // C               # w-blocks per chunk
NCH = WC // F             # chunks per image
TOT = BS * NCH            # chunks per core
EXT = F + 2 * C           # chunk + one w-block halo on each side
NBUF = 2                  # buffer depth for q/out tiles; lane sems ci % NBUF
MMF = 512                 # max fp32 matmul moving free dim (one PSUM bank)

_FP = mybir.dt.float32
_MUL = mybir.AluOpType.mult
_ADD = mybir.AluOpType.add


def _chunks():
    return [(b * NCH + k, b, k) for b in range(BS) for k in range(NCH)]


def h_weights():
    """lhsT (stationary, [K=in_row, M=out_partition]) for the two H phases."""
    we = np.zeros((H, H), dtype=np.float32)   # qE[m] = out row 2m, = row/4
    i = np.arange(H)
    we[i, i] = 0.1875                          # 3/16
    we[0, 0] = 0.25                            # edge clamp: 4/16
    we[i[:-1], i[:-1] + 1] = 0.0625            # cur[m-1] term: k == m-1
    wo = np.zeros((H, H), dtype=np.float32)   # qO[m] = out row 2m+1
    wo[i, i] = 0.1875
    wo[H - 1, H - 1] = 0.25
    wo[i[1:], i[1:] - 1] = 0.0625              # cur[m+1] term: k == m+1
    return we, wo


def _mm_pieces():
    """(c0, c1) col pieces of EXT, each within one PSUM bank."""
    out = []
    c = 0
    while c < EXT:
        out.append((c, min(c + MMF, EXT)))
        c += MMF
    return out


def _build(**bass_kwargs):
    nc = bass.Bass(**bass_kwargs)
    x = nc.dram_tensor("x", [BS, H, WC], _FP, kind="ExternalInput")
    we_d = nc.dram_tensor("we", [H, H], _FP, kind="ExternalInput")
    wo_d = nc.dram_tensor("wo", [H, H], _FP, kind="ExternalInput")
    y = nc.dram_tensor("y", [BS, 2 * H, 2 * WC], _FP, kind="ExternalOutput")

    chunks = _chunks()
    pieces = _mm_pieces()
    NMM = len(pieces)           # matmuls per phase per chunk

    def st_cnt(ci):             # store DMAs on lane sem through chunk ci
        return 2 * (ci // NBUF + 1)

    with ExitStack() as ctx:
        def sb(nm, width):
            return ctx.enter_context(nc.sbuf_tensor(nm, [128, width], _FP))

        img = [sb(f"img{i}", 2 * C + WC) for i in range(BS)]
        qe = [sb(f"qe{i}", EXT) for i in range(NBUF)]
        qo = [sb(f"qo{i}", EXT) for i in range(NBUF)]
        outt = [sb(f"outt{i}", 4 * F) for i in range(NBUF)]
        we_sb = sb("we_sb", H)
        wo_sb = sb("wo_sb", H)
        # 1536 cols = 3 whole PSUM banks each, so every 512-col matmul piece
        # sits inside a single bank
        qe_ps = ctx.enter_context(nc.psum_tensor("qe_ps", [128, 1536], _FP))
        qo_ps = ctx.enter_context(nc.psum_tensor("qo_ps", [128, 1536], _FP))

        sem = lambda nm: ctx.enter_context(nc.semaphore(nm))
        s_in = [sem(f"s_in{i}") for i in range(2 * BS)]
        s_out = [sem(f"s_out{i}") for i in range(NBUF)]
        s_w = sem("s_w")
        s_pe = sem("s_pe")
        s_cp = sem("s_cp")
        s_dve = sem("s_dve")
        s_fin = sem("s_fin")
        all_sems = s_in + s_out + [s_w, s_pe, s_cp, s_dve, s_fin]

        block = ctx.enter_context(nc.Block())

        @block.sync
        def _(sync):
            sync.dma_start(out=we_sb[:], in_=we_d[:]).then_inc(s_w, 16)
            sync.dma_start(out=wo_sb[:], in_=wo_d[:]).then_inc(s_w, 16)
            # whole image + duplicated first/last w-block (W edge clamp),
            # split into two column halves so PE can start after the first
            # half lands (halves the pipeline-fill exposure)
            HSPL = 7 * F + EXT          # tile-coord split; chunks 0..7 use half 1
            for b in range(BS):
                sync.dma_start(out=img[b][:, C:HSPL], in_=x[b][:, 0 : HSPL - C]).then_inc(s_in[2 * b], 16)
                sync.dma_start(out=img[b][:, 0:C], in_=x[b][:, 0:C]).then_inc(s_in[2 * b], 16)
                sync.dma_start(out=img[b][:, HSPL : C + WC], in_=x[b][:, HSPL - C : WC]).then_inc(s_in[2 * b + 1], 16)
                sync.dma_start(out=img[b][:, C + WC :], in_=x[b][:, WC - C : WC]).then_inc(s_in[2 * b + 1], 16)
            for ci, b, k in chunks:
                l = ci % NBUF
                cols = slice(2 * k * F, 2 * (k + 1) * F)
                # even rows as soon as DVE ops 1,2 are done; odd after 3,4
                sync.wait_ge(s_dve, 4 * ci + 2)
                sync.dma_start(
                    out=y[b][0 : 2 * H : 2, cols], in_=outt[l][:, 0 : 2 * F]
                ).then_inc(s_out[l], 16)
                sync.wait_ge(s_dve, 4 * ci + 4)
                sync.dma_start(
                    out=y[b][1 : 2 * H : 2, cols], in_=outt[l][:, 2 * F : 4 * F]
                ).then_inc(s_out[l], 16)
            # ---- finish: all stores landed, all engines idle, reset sems
            for l in range(NBUF):
                last = TOT - 1 - ((TOT - 1 - l) % NBUF)
                sync.wait_ge(s_out[l], 16 * st_cnt(last))
            sync.wait_ge(s_fin, 3)
            for s in all_sems:
                sync.sem_clear(s)

        @block.tensor
        def _(pe):
            pe.wait_ge(s_w, 32)
            for ci, b, k in chunks:
                pe.wait_ge(s_in[2 * b], 32)
                if k >= 8:
                    pe.wait_ge(s_in[2 * b + 1], 32)
                if ci >= 1:
                    # qe_ps reader (ACT E-copy of chunk ci-1) must be done
                    pe.wait_ge(s_cp, 2 * (ci - 1) + 1)
                rhs = img[b][:, k * F : k * F + EXT]
                for c0, c1 in pieces:
                    pe.matmul(
                        out=qe_ps[:, c0:c1], lhsT=we_sb[:], rhs=rhs[:, c0:c1],
                        start=True, stop=True,
                    ).then_inc(s_pe, 1)
                if ci >= 1:
                    pe.wait_ge(s_cp, 2 * (ci - 1) + 2)
                for c0, c1 in pieces:
                    pe.matmul(
                        out=qo_ps[:, c0:c1], lhsT=wo_sb[:], rhs=rhs[:, c0:c1],
                        start=True, stop=True,
                    ).then_inc(s_pe, 1)
            pe.sem_inc(s_fin, 1)

        @block.scalar
        def _(act):
            for ci, b, k in chunks:
                l = ci % NBUF
                act.wait_ge(s_pe, 2 * NMM * ci + NMM)
                if ci >= NBUF:
                    # qe[l] readers (DVE ops 1,2 of chunk ci-NBUF) must be done
                    act.wait_ge(s_dve, 4 * (ci - NBUF) + 2)
                act.activation(
                    qe[l][:], qe_ps[:, 0:EXT], mybir.ActivationFunctionType.Copy,
                ).then_inc(s_cp, 1)
                act.wait_ge(s_pe, 2 * NMM * ci + 2 * NMM)
                if ci >= NBUF:
                    act.wait_ge(s_dve, 4 * (ci - NBUF) + 4)
                act.activation(
                    qo[l][:], qo_ps[:, 0:EXT], mybir.ActivationFunctionType.Copy,
                ).then_inc(s_cp, 1)
            act.sem_inc(s_fin, 1)

        @block.vector
        def _(vec):
            for ci, b, k in chunks:
                l = ci % NBUF
                vec.wait_ge(s_cp, 2 * ci + 1)
                if ci >= NBUF:
                    vec.wait_ge(s_out[l], 16 * st_cnt(ci - NBUF))
                qev = qe[l][:].rearrange("p (a c) -> p a c", c=C)
                qov = qo[l][:].rearrange("p (a c) -> p a c", c=C)
                ov = outt[l][:].rearrange("p (t a u c) -> p t a u c", t=2, u=2, c=C)
                vec.scalar_tensor_tensor(
                    ov[:, 0, :, 0, :], qev[:, 1 : NW + 1, :], 3.0,
                    qev[:, 0:NW, :], _MUL, _ADD,
                ).then_inc(s_dve, 1)
                vec.scalar_tensor_tensor(
                    ov[:, 0, :, 1, :], qev[:, 1 : NW + 1, :], 3.0,
                    qev[:, 2 : NW + 2, :], _MUL, _ADD,
                ).then_inc(s_dve, 1)
                vec.wait_ge(s_cp, 2 * ci + 2)
                vec.scalar_tensor_tensor(
                    ov[:, 1, :, 0, :], qov[:, 1 : NW + 1, :], 3.0,
                    qov[:, 0:NW, :], _MUL, _ADD,
                ).then_inc(s_dve, 1)
                vec.scalar_tensor_tensor(
                    ov[:, 1, :, 1, :], qov[:, 1 : NW + 1, :], 3.0,
                    qov[:, 2 : NW + 2, :], _MUL, _ADD,
                ).then_inc(s_dve, 1)
            vec.sem_inc(s_fin, 1)

    return nc


_NC = None


def kernel(inputs: np.ndarray) -> np.ndarray:
    global _NC
    assert inputs.shape == (B, H, W, C), inputs.shape
    x = np.ascontiguousarray(inputs, dtype=np.float32).reshape(B, H, WC)
    if _NC is None:
        _NC = _build()
    we, wo = h_weights()
    in_maps = [
        {"x": x[i * BS : (i + 1) * BS], "we": we, "wo": wo} for i in range(NCORES)
    ]
    res = run_bass_kernel_spmd(_NC, in_maps, list(range(NCORES))).results
    out = np.empty((B, 2 * H, 2 * W, C), dtype=np.float32)
    for i in range(NCORES):
        out[i * BS : (i + 1) * BS] = res[i]["y"].reshape(BS, 2 * H, 2 * W, C)
    return out
